# revision 20
# baseline (speedup 1.0000x reference)
"""Multi-Latent Attention TRN2 kernel.

Sharding: tensor-parallel over heads. 16 heads / 8 cores = 2 heads per core.
Each core computes its 2 heads' projections + attention and a partial of the
final output projection (contracting only its heads' feature block); the host
sums the 8 partials and adds the output bias.

On-device dataflow is feature-major (transposed): the host feeds X^T for
queries/keys/values so every matmul contracts along SBUF partitions.

  q^T   = Wq_c^T  X_q^T            [256, T]
  latk^T= Wlk_c^T X_k^T            [128, T]
  latv^T= Wlv_c^T X_v^T            [128, T]
  k^T   = blockdiag(Wkr)^T latk^T  [256, T]  (per head)
  v     = latv blockdiag(Wvr)      [T, 256]  (token-major)
  P~^T  = exp(k q^T / sqrt(dk))    (S^T computed directly; no transposes)
  rowsum= ones^T P~^T              (ones-vector matmul)
  U^T   = v^T P~^T
  attnout^T = U^T * recip(rowsum) + bvr
  out_partial = attnout @ Wo_rows

Softmax skips the max-subtraction: scores are O(1) by construction
(inputs ~N(0,1), 1/sqrt(fan_in)-scaled weights), so exp cannot overflow.
"""

import math
from contextlib import ExitStack

import numpy as np

import concourse.mybir as mybir
from concourse import bacc
from concourse.bass import ds, ts
from concourse.tile import TileContext

# Problem constants (hardcoded per contract).
B, S, D = 2, 2048, 2048
H, DK, DV, L = 16, 128, 128, 64
N_CORES = 8
HPC = H // N_CORES        # heads per core = 2
T = B * S                 # 4096 tokens
SB = S                    # tokens per batch
FPC = HPC * DK            # feature cols per core = 256
LPC = HPC * L             # latent cols per core = 128
KO = D // 128             # contraction k-tiles over D = 16
QT = SB // 128            # 128-row tiles per batch = 16
NQB = SB // 512           # 512-wide q blocks per batch = 4

F32 = mybir.dt.float32
F32R = mybir.dt.float32r
BF16 = mybir.dt.bfloat16

# dtype switches
IN_BF16 = True           # stream X^T (and proj weights) as bf16
OUT_BF16 = True          # write output partials as bf16

IN_DT = BF16 if IN_BF16 else F32R
OUT_DT = BF16 if OUT_BF16 else F32
CHUNK = 512 if IN_BF16 else 256
NCH = SB // CHUNK

INV_SQRT_DK = 1.0 / math.sqrt(DK)
EXPF = mybir.ActivationFunctionType.Exp
IDF = mybir.ActivationFunctionType.Identity


def build_kernel():
    nc = bacc.Bacc(trn_type="TRN2", debug=False, num_swdge_queues=2)

    # ---- DRAM I/O ----
    qT = nc.dram_tensor("qT", [D, T], IN_DT, kind="ExternalInput")
    kT = nc.dram_tensor("kT", [D, T], IN_DT, kind="ExternalInput")
    vT = nc.dram_tensor("vT", [D, T], IN_DT, kind="ExternalInput")
    wq = nc.dram_tensor("wq", [D, FPC], IN_DT, kind="ExternalInput")
    bq = nc.dram_tensor("bq", [FPC], F32, kind="ExternalInput")
    wlk = nc.dram_tensor("wlk", [D, LPC], IN_DT, kind="ExternalInput")
    blk = nc.dram_tensor("blk", [LPC], F32, kind="ExternalInput")
    wlv = nc.dram_tensor("wlv", [D, LPC], IN_DT, kind="ExternalInput")
    blv = nc.dram_tensor("blv", [LPC], F32, kind="ExternalInput")
    wkr2 = nc.dram_tensor("wkr2", [LPC, FPC], F32R, kind="ExternalInput")
    bkr = nc.dram_tensor("bkr", [DK], F32, kind="ExternalInput")
    wvr2 = nc.dram_tensor("wvr2", [LPC, FPC], F32R, kind="ExternalInput")
    bvr = nc.dram_tensor("bvr", [DV], F32, kind="ExternalInput")
    wo = nc.dram_tensor("wo", [FPC, D], BF16, kind="ExternalInput")
    outp = nc.dram_tensor("outp", [T, D], OUT_DT, kind="ExternalOutput")

    with TileContext(nc) as tc, ExitStack() as ctx:
        ec = ctx.enter_context
        consts = ec(tc.tile_pool(name="consts", bufs=1))
        persist = ec(tc.tile_pool(name="persist", bufs=1))
        xpool = ec(tc.tile_pool(name="xpool", bufs=3))
        latpool = ec(tc.tile_pool(name="latpool", bufs=3))
        ptpool = ec(tc.tile_pool(name="ptpool", bufs=2))
        statpool = ec(tc.tile_pool(name="statpool", bufs=4))
        opool = ec(tc.tile_pool(name="opool", bufs=3))
        psa = ec(tc.tile_pool(name="psa", bufs=2, space="PSUM"))
        pss = ec(tc.tile_pool(name="pss", bufs=2, space="PSUM"))
        pso = ec(tc.tile_pool(name="pso", bufs=2, space="PSUM"))
        psu = ec(tc.tile_pool(name="psu", bufs=2, space="PSUM"))

        # ---- constants / weights ----
        # causal mask for a diagonal 128x128 block of P~^T: 1 where k <= q
        # (partition = k, free = q)
        maskT = consts.tile([128, 128], BF16, tag="maskT")
        nc.gpsimd.memset(maskT, 1.0)
        nc.gpsimd.affine_select(
            out=maskT, in_=maskT, compare_op=mybir.AluOpType.is_ge,
            fill=0.0, base=0, pattern=[[1, 128]], channel_multiplier=-1,
        )
        ones_bf = consts.tile([128, 128], BF16, tag="ones_bf")
        nc.gpsimd.memset(ones_bf, 1.0)

        wq_sb = consts.tile([128, KO, FPC], IN_DT, tag="wq")
        nc.sync.dma_start(wq_sb, wq.rearrange("(ko p) m -> p ko m", p=128))
        wlk_sb = consts.tile([128, KO, LPC], IN_DT, tag="wlk")
        nc.gpsimd.dma_start(wlk_sb, wlk.rearrange("(ko p) m -> p ko m", p=128))
        wlv_sb = consts.tile([128, KO, LPC], IN_DT, tag="wlv")
        nc.gpsimd.dma_start(wlv_sb, wlv.rearrange("(ko p) m -> p ko m", p=128))
        wkr2_sb = consts.tile([128, FPC], F32R, tag="wkr2")
        nc.gpsimd.dma_start(wkr2_sb, wkr2[:, :])
        wvr2_sb = consts.tile([128, FPC], F32R, tag="wvr2")
        nc.gpsimd.dma_start(wvr2_sb, wvr2[:, :])

        wo_sb = consts.tile([128, HPC, D], BF16, tag="wo")

        bq_sb = consts.tile([128, HPC], F32, tag="bq")
        nc.gpsimd.dma_start(bq_sb, bq.rearrange("(m p) -> p m", p=128))
        blk_sb = consts.tile([128, 1], F32, tag="blk")
        nc.gpsimd.dma_start(blk_sb, blk[:, None])
        blv_sb = consts.tile([128, 1], F32, tag="blv")
        nc.gpsimd.dma_start(blv_sb, blv[:, None])
        bkr_sb = consts.tile([128, 1], F32, tag="bkr")
        nc.gpsimd.dma_start(bkr_sb, bkr[:, None])
        bvr_sb = consts.tile([128, 1], F32, tag="bvr")
        nc.gpsimd.dma_start(bvr_sb, bvr[:, None])

        # attnout^T (both batches), feature-major, lhsT of final matmul
        asb = persist.tile([128, HPC, T], BF16, tag="asb")

        qT_r = qT.rearrange("(ko p) t -> p ko t", p=128)
        kT_r = kT.rearrange("(ko p) t -> p ko t", p=128)
        vT_r = vT.rearrange("(ko p) t -> p ko t", p=128)

        for b in range(B):
            qsb = persist.tile([128, HPC, SB], BF16, tag=f"qsb{b}")
            ksb = persist.tile([128, HPC, SB], BF16, tag=f"ksb{b}")
            vsb = persist.tile([128, QT, FPC], BF16, tag=f"vsb{b}")

            # ---- projections, streamed over token chunks ----
            for c in range(NCH):
                t0 = b * SB + c * CHUNK  # global token start
                csl = ds(c * CHUNK, CHUNK)

                # q^T chunk
                xq = xpool.tile([128, KO, CHUNK], IN_DT, tag="x")
                nc.sync.dma_start(xq, qT_r[:, :, ds(t0, CHUNK)])
                for m in range(HPC):
                    for n2 in range(CHUNK // 256):
                        ps = psa.tile([128, 512], F32, tag="s")
                        for ko in range(KO):
                            nc.tensor.matmul(
                                ps[:, :256],
                                wq_sb[:, ko, ts(m, 128)],
                                xq[:, ko, ts(n2, 256)],
                                start=(ko == 0), stop=(ko == KO - 1),
                            )
                        nc.vector.tensor_scalar_add(
                            qsb[:, m, ds(c * CHUNK + n2 * 256, 256)],
                            ps[:, :256], bq_sb[:, m : m + 1],
                        )

                # latk chunk -> k^T chunk (per head)
                xk = xpool.tile([128, KO, CHUNK], IN_DT, tag="x")
                nc.sync.dma_start(xk, kT_r[:, :, ds(t0, CHUNK)])
                for n2 in range(CHUNK // 256):
                    lk = latpool.tile([128, 256], F32R, tag="lat")
                    ps = psa.tile([128, 512], F32, tag="s")
                    for ko in range(KO):
                        nc.tensor.matmul(
                            ps[:, :256], wlk_sb[:, ko, :],
                            xk[:, ko, ts(n2, 256)],
                            start=(ko == 0), stop=(ko == KO - 1),
                        )
                    nc.scalar.activation(lk, ps[:, :256], IDF,
                                         bias=blk_sb[:, 0:1])
                    for h in range(HPC):
                        psk = psa.tile([128, 512], F32, tag="s")
                        nc.tensor.matmul(
                            psk[:, :256], wkr2_sb[:, ts(h, 128)], lk,
                            start=True, stop=True,
                        )
                        nc.scalar.activation(
                            ksb[:, h, ds(c * CHUNK + n2 * 256, 256)],
                            psk[:, :256], IDF, bias=bkr_sb[:, 0:1],
                        )

                # latv chunk -> v (token-major) chunk
                xv = xpool.tile([128, KO, CHUNK], IN_DT, tag="x")
                nc.gpsimd.dma_start(xv, vT_r[:, :, ds(t0, CHUNK)])
                for n2 in range(CHUNK // 256):
                    lv = latpool.tile([128, 256], F32R, tag="lat")
                    ps = psa.tile([128, 512], F32, tag="s")
                    for ko in range(KO):
                        nc.tensor.matmul(
                            ps[:, :256], wlv_sb[:, ko, :],
                            xv[:, ko, ts(n2, 256)],
                            start=(ko == 0), stop=(ko == KO - 1),
                        )
                    nc.scalar.activation(lv, ps[:, :256], IDF,
                                         bias=blv_sb[:, 0:1])
                    for j2 in range(2):
                        psv = psa.tile([128, 512], F32, tag="s")
                        nc.tensor.matmul(
                            psv[:, :FPC], lv[:, ts(j2, 128)], wvr2_sb,
                            start=True, stop=True,
                        )
                        jt = (c * CHUNK + n2 * 256) // 128 + j2
                        nc.any.tensor_copy(out=vsb[:, jt, :],
                                           in_=psv[:, :FPC])

            if b == 0:
                nc.gpsimd.dma_start(
                    wo_sb, wo.rearrange("(kk p) d -> p kk d", p=128)
                )

            # ---- attention + final projection, per 512-wide q block ----
            for Q in range(NQB):
                for h in range(HPC):
                    jmax = 4 * Q + 4          # k-tiles 0..jmax-1
                    ptq = ptpool.tile([128, QT, 512], BF16, tag="pt")

                    for j in range(jmax):
                        qoff = max(0, (j - 4 * Q) * 128)
                        n = 512 - qoff
                        ps_s = pss.tile([128, 512], F32, tag="st")
                        nc.tensor.matmul(
                            ps_s[:, :n], ksb[:, h, ts(j, 128)],
                            qsb[:, h, ds(Q * 512 + qoff, n)],
                            start=True, stop=True,
                        )
                        nc.scalar.activation(
                            ptq[:, j, ds(qoff, n)], ps_s[:, :n],
                            EXPF, scale=INV_SQRT_DK,
                        )
                        if j >= 4 * Q:  # diagonal k-tile: causal mask
                            nc.vector.tensor_tensor(
                                ptq[:, j, ds(qoff, 128)],
                                ptq[:, j, ds(qoff, 128)],
                                maskT, mybir.AluOpType.mult,
                            )

                    # row sums of P~ (per q), replicated across all 128
                    # partitions via a full ones matrix as lhsT
                    ps_o = pso.tile([128, 512], F32, tag="o")
                    for j in range(jmax):
                        qoff = max(0, (j - 4 * Q) * 128)
                        nc.tensor.matmul(
                            ps_o[:, qoff:], ones_bf, ptq[:, j, qoff:],
                            start=(j == 0), stop=(j == jmax - 1),
                        )
                    rcp_sb = statpool.tile([128, 512], F32, tag="rcp")
                    nc.vector.reciprocal(rcp_sb, ps_o)

                    # U^T = v^T P~^T
                    ps_u = psu.tile([128, 512], F32, tag="u")
                    for j in range(jmax):
                        qoff = max(0, (j - 4 * Q) * 128)
                        nc.tensor.matmul(
                            ps_u[:, qoff:], vsb[:, j, ts(h, 128)],
                            ptq[:, j, qoff:],
                            start=(j == 0), stop=(j == jmax - 1),
                        )

                    a_sl = asb[:, h, ds(b * SB + Q * 512, 512)]
                    nc.vector.tensor_tensor(a_sl, ps_u, rcp_sb,
                                            mybir.AluOpType.mult)
                    nc.vector.tensor_scalar_add(a_sl, a_sl, bvr_sb[:, 0:1])

                # final projection for this q-block's 4 token tiles
                for tl in range(4):
                    tt = b * QT + Q * 4 + tl
                    o_sb = opool.tile([128, D], OUT_DT, tag="o")
                    for dc in range(D // 512):
                        ps_f = psa.tile([128, 512], F32, tag="s")
                        for kk in range(HPC):
                            nc.tensor.matmul(
                                ps_f, asb[:, kk, ts(tt, 128)],
                                wo_sb[:, kk, ts(dc, 512)],
                                start=(kk == 0), stop=(kk == HPC - 1),
                            )
                        nc.any.tensor_copy(out=o_sb[:, ts(dc, 512)], in_=ps_f)
                    nc.sync.dma_start(outp[ts(tt, 128), :], o_sb)


    nc.finalize()
    return nc


_NC_CACHE = None


def _get_nc():
    global _NC_CACHE
    if _NC_CACHE is None:
        _NC_CACHE = build_kernel()
    return _NC_CACHE


def _prep_in_maps(queries, keys, values, Wq, bq, Wlk, blk, Wlv, blv,
                  Wkr, bkr, Wvr, bvr, Wo, bo):
    f = np.float32
    import ml_dtypes

    ind = ml_dtypes.bfloat16 if IN_BF16 else f

    qTh = np.ascontiguousarray(queries.reshape(T, D).T.astype(ind))
    kTh = np.ascontiguousarray(keys.reshape(T, D).T.astype(ind))
    vTh = np.ascontiguousarray(values.reshape(T, D).T.astype(ind))

    wkr2 = np.zeros((LPC, FPC), f)
    wkr2[0:L, 0:DK] = Wkr
    wkr2[L : 2 * L, DK : 2 * DK] = Wkr
    wvr2 = np.zeros((LPC, FPC), f)
    wvr2[0:L, 0:DV] = Wvr
    wvr2[L : 2 * L, DV : 2 * DV] = Wvr

    in_maps = []
    for c in range(N_CORES):
        fsl = slice(c * FPC, (c + 1) * FPC)   # feature cols (q/k heads)
        lsl = slice(c * LPC, (c + 1) * LPC)   # latent cols
        in_maps.append({
            "qT": qTh, "kT": kTh, "vT": vTh,
            "wq": np.ascontiguousarray(Wq[:, fsl].astype(ind)),
            "bq": np.ascontiguousarray(bq[fsl], f),
            "wlk": np.ascontiguousarray(Wlk[:, lsl].astype(ind)),
            "blk": np.ascontiguousarray(blk[lsl], f),
            "wlv": np.ascontiguousarray(Wlv[:, lsl].astype(ind)),
            "blv": np.ascontiguousarray(blv[lsl], f),
            "wkr2": wkr2, "bkr": np.ascontiguousarray(bkr, f),
            "wvr2": wvr2, "bvr": np.ascontiguousarray(bvr, f),
            "wo": np.ascontiguousarray(Wo[fsl, :].astype(ml_dtypes.bfloat16)),
        })
    return in_maps


def _assemble(results, bo):
    acc = np.zeros((T, D), np.float64)
    for rmap in results:
        acc += rmap["outp"].astype(np.float64)
    acc += np.asarray(bo).astype(np.float64)
    return acc.astype(np.float32).reshape(B, S, D)


def kernel(**inputs):
    from concourse.bass_utils import run_bass_kernel_spmd

    nc = _get_nc()
    in_maps = _prep_in_maps(**inputs)
    res = run_bass_kernel_spmd(
        nc, in_maps, core_ids=list(range(N_CORES)), trace=False
    )
    return _assemble(res.results, inputs["bo"])


if __name__ == "__main__":
    nc = build_kernel()
    print("built ok, instructions:", len(nc.inst_map))
